# revision 29
# baseline (speedup 1.0000x reference)
"""Trainium2 Bass kernel for fused BERT-CRF-NER word_embedding + sigmoid.

Math (per batch row):
  inner[t]   = 1 <= t <= L-2          (L = valid length from contiguous mask)
  starts     = first_label_mask & inner
  wid2[t]    = cumsum(starts) * inner (1-based word id, 0 outside inner)
  wv[k]      = mean of token_features[t] over wid2[t] == k+1
  emission   = sigmoid(wv @ W.T + b)  (empty word slots -> sigmoid(b))

Restructuring for the hardware:
  1) membership matrix M[t, k] = (wid2[t] == k+1)            [128-chunk, K]
  2) Z^T[d, k]  = sum_t X[t, d] M[t, k]     (PE, X chunks stationary in the
     natural [t, d] layout -> X is never transposed)
  3) lg^T[l, k] = sum_d W^T[d, l] Z^T[d, k] + b[l]*max(cnt[k], 1)
     (bias folded in as a rank-1 matmul so (lg/cnt) = logits + b exactly,
      and empty slots come out as sigmoid(b) for free)
  4) transpose lg^T together with a stacked 1/cnt row (11-row bf16
     transpose), then one fused ACT op per column group:
     sigmoid(lg * recip_scale)
  5) one 160B-per-partition output store per row (word slots p-major)

Ragged specialization: lengths vary 16..512, so the host sorts rows by
length and deals them round-robin to the 8 cores (slot j on every core
holds rows of similar length).  Per-slot token-chunk count TC[j] and
word capacity K4[j] are derived from the actual masks at runtime and
baked into the compiled program (cached per (TC, K4) tuple).  This cuts
both HBM traffic and PE work ~40% vs processing full 512-token rows.

Heavy matmuls run in bf16 (X cast fp32->bf16 during the SWDGE DMA),
accumulation in fp32 PSUM.  All PE transposes use bf16 stationaries
(fp32 pays a double-pass LDWEIGHTS).  Sharding: data parallel, 8
rows/core.
"""

from contextlib import ExitStack

import numpy as np

import concourse.bass as bass
import concourse.tile as tile
from concourse import bacc, mybir
from concourse.bass_utils import run_bass_kernel_spmd

B, S, D, NL = 64, 512, 768, 10
N_CORES = 8
RPC = B // N_CORES  # batch rows (slots) per core
DC = D // 128       # feature chunks of 128

f32 = mybir.dt.float32
bf16 = mybir.dt.bfloat16
i32 = mybir.dt.int32
Alu = mybir.AluOpType
Act = mybir.ActivationFunctionType


def _plan(input_mask, first_label_mask):
    """Host-side integer metadata: row->slot assignment and per-slot caps."""
    im = np.asarray(input_mask, np.int64)
    fm = np.asarray(first_label_mask, np.int64)
    L = im.sum(1)
    pos = np.arange(S)
    inner = (im > 0) & (pos[None, :] >= 1) & (pos[None, :] <= (L - 2)[:, None])
    words = ((fm > 0) & inner).sum(1)
    order = np.argsort(-L, kind="stable")  # slot j, core i -> order[j*8+i]
    TC, K4, CAP = [], [], []
    for j in range(RPC):
        rows = order[j * N_CORES : (j + 1) * N_CORES]
        cap = max(1, int(L[rows].max()))
        CAP.append(cap)
        TC.append(-(-cap // 128))
        K4.append(max(4, -(-int(words[rows].max()) // 4) * 4))
    return order, tuple(TC), tuple(K4), tuple(CAP)


def _build_nc(TC, K4, CAP):
    KM = max(K4)
    assert max(TC) <= S // 128 and KM <= 256
    nc = bacc.Bacc("TRN2", target_bir_lowering=False, debug=False)
    x_d = nc.dram_tensor("x", [RPC, S, D], f32, kind="ExternalInput")
    msk_d = nc.dram_tensor("msk", [RPC, 2 * S], i32, kind="ExternalInput")
    wt_d = nc.dram_tensor("wt", [128, DC * NL], f32, kind="ExternalInput")
    b_d = nc.dram_tensor("b", [1, NL], f32, kind="ExternalInput")
    # one merged bf16 const blob: [0:16]=identity, [16]=ones col,
    # [17:145]=ones row, [145:145+KM]=iota 1..KM  (one DMA, one sem)
    cb_d = nc.dram_tensor("cb", [128, 145 + KM], bf16, kind="ExternalInput")
    out_d = nc.dram_tensor("out", [RPC, S, NL], f32, kind="ExternalOutput")

    with tile.TileContext(nc) as tc, ExitStack() as ctx:
        const = ctx.enter_context(tc.tile_pool(name="const", bufs=1))
        xp = ctx.enter_context(tc.tile_pool(name="xp", bufs=sum(TC)))
        mp = ctx.enter_context(tc.tile_pool(name="mp", bufs=10))
        zsp = ctx.enter_context(tc.tile_pool(name="zsp", bufs=2))
        rsp = ctx.enter_context(tc.tile_pool(name="rsp", bufs=2))
        obp = ctx.enter_context(tc.tile_pool(name="obp", bufs=2))
        ztp = ctx.enter_context(
            tc.tile_pool(name="ztp", bufs=2, space=bass.MemorySpace.PSUM)
        )
        lgp = ctx.enter_context(
            tc.tile_pool(name="lgp", bufs=1, space=bass.MemorySpace.PSUM)
        )
        ctp = ctx.enter_context(
            tc.tile_pool(name="ctp", bufs=1, space=bass.MemorySpace.PSUM)
        )
        tpp = ctx.enter_context(
            tc.tile_pool(name="tpp", bufs=2, space=bass.MemorySpace.PSUM)
        )

        # ---- X chunk loads first: they pace the whole kernel ------------
        # One SWDGE cast-DMA (fp32 -> bf16) per 128-token chunk; both DRAM
        # and SBUF sides are one contiguous run per partition.  Issue order
        # == PE consumption order (slots descending by length).
        xs = {}
        for j in range(RPC):
            for c in range(TC[j]):
                x_t = xp.tile([128, D], bf16, tag="x", name=f"x{j}_{c}")
                lo = c * 128
                n = min(128, CAP[j] - lo)  # exact cap: skip invalid tail rows
                if n < 128:
                    # pad region must be zero (M is 0 there, but 0*garbage
                    # could be NaN).  Emitted BEFORE the dma_start so the
                    # partial-chunk DMA overwrites the overlapped rows, not
                    # the other way around.  DVE is idle this early; 32-
                    # partition blocks (non-zero starts allow at most 32).
                    for q in range((n // 32) * 32, 128, 32):
                        nc.vector.memset(x_t[q : q + 32, :], 0.0)
                nc.gpsimd.dma_start(x_t[0:n, :], x_d[j, lo : lo + n, :])
                xs[(j, c)] = x_t

        # ---- small loads on the HWDGE sync queue (mask-chain deps first)
        msk_i = const.tile([RPC, 2 * S], i32)
        nc.sync.dma_start(msk_i[:], msk_d[:, :])
        wt_f = const.tile([128, DC * NL], f32)  # host-permuted W^T
        nc.sync.dma_start(wt_f[:], wt_d[:, :])
        b_sb = const.tile([1, NL], f32)
        nc.sync.dma_start(b_sb[:], b_d[:, :])
        # scalar HWDGE queue: separate ring, so this doesn't queue behind
        # Q1 packets starved by the saturated X stream; one DMA so no tiny
        # straggler const can stall the PE schedule
        cb_t = const.tile([128, 145 + KM], bf16)
        nc.scalar.dma_start(cb_t[:], cb_d[:, :])
        ci_t = cb_t[0:16, 0:16]    # identity for PE transposes
        cr_t = cb_t[:, 16:17]      # ones column (counts matmul)
        c1_t = cb_t[0:1, 17:145]   # ones row (partition broadcast)
        ck_t = cb_t[:, 145 : 145 + KM]  # iota 1..KM (exact in bf16)

        # late consts (emitted after slot 0 below): bias + sigmoid(b) bcast
        b16 = const.tile([1, NL], bf16)
        wt16 = const.tile([128, DC * NL], bf16)
        wt_v = wt16[:].rearrange("p (j l) -> p j l", l=NL)
        sigb_row = const.tile([1, NL], bf16)
        sigb2 = const.tile([128, 2, NL], f32)
        sigb_big = const.tile([128, RPC, 2, NL], f32)

        def setup_late_consts():
            # on scalar, NOT vector: a vector-queue cast waiting on the wt/b
            # DMAs would block every later M-build behind it
            nc.scalar.copy(b16[:], b_sb[:])
            nc.scalar.copy(wt16[:], wt_f[:])
            with nc.allow_low_precision(reason="bf16 sigmoid(b), tol 2e-2"):
                nc.scalar.activation(sigb_row[:], b_sb[:], Act.Sigmoid)
            sb_ps = lgp.tile([128, 16], f32, tag="lgp")
            nc.tensor.matmul(sb_ps[:, 0:NL], c1_t, sigb_row[0:1, :])
            for c in range(2):
                nc.scalar.copy(sigb2[:, c, :], sb_ps[:, 0:NL])
            # word slots 256..511 are sigmoid(b) for every row (max words
            # ~206): one early DMA covers them all, off the critical path
            for r in range(RPC):
                nc.scalar.copy(sigb_big[:, r, :, :], sigb2[:])
            nc.scalar.dma_start(
                out_d[:, 256:512, :].rearrange("r (q c) l -> q r c l", c=2),
                sigb_big[:],
            )

        setup_late_consts()

        # ---- HAM warm-up: ~3.5us of dummy matmuls while PE waits for the
        # mask chain; keeps the PE clock at 2.4GHz for the real work
        NW = 145 + KM
        warm_ps = lgp.tile([128, NW], f32, tag="lgp", name="warm")
        for _ in range(12):
            nc.tensor.matmul(
                warm_ps[:, 0:NW], cb_t[:, 0:128], cb_t[:, 0:NW],
                start=True, stop=True,
            )

        # ---- mask pipeline: 1-based word ids, all RPC rows at once ------
        # contiguous mask => inner[t] = (1 <= t <= L-2) = im[t+1] (t>=1)
        im_v = msk_i[:, 0:S]
        fm_v = msk_i[:, S : 2 * S]
        inner = const.tile([RPC, S], f32)
        nc.vector.tensor_copy(inner[:, 1 : S - 1], im_v[:, 2:S])
        nc.vector.memset(inner[:, 0:1], 0.0)
        nc.vector.memset(inner[:, S - 1 : S], 0.0)
        starts = const.tile([RPC, S], f32)
        nc.vector.tensor_mul(starts[:], fm_v, inner[:])
        widr = const.tile([RPC, S], f32)
        nc.vector.tensor_tensor_scan(
            widr[:], starts[:], starts[:], 0.0, op0=Alu.add, op1=Alu.bypass
        )
        wid2 = const.tile([RPC, S], bf16)
        nc.vector.tensor_mul(wid2[:], widr[:], inner[:])

        # transpose word ids onto token partitions (chunk-major: t = c*128+p)
        TCM = max(TC)
        widT = const.tile([128, TCM, RPC], f32)  # is_equal scalar must be f32
        wid_v = wid2[:].rearrange("r (c p) -> r c p", p=128)
        for c in range(TCM):
            tp_ps = tpp.tile([128, 16], bf16, tag="tp")
            nc.tensor.transpose(tp_ps[:, 0:RPC], wid_v[:, c, :], ci_t[0:RPC, 0:RPC])
            nc.vector.tensor_copy(widT[:, c, :], tp_ps[:, 0:RPC])


        # ---- heavy per-slot pipeline, software-pipelined ----------------
        # Slots are processed in pairs: stage-2 (logits) matmuls and the
        # PSUM->SBUF logits copy run once per pair over the concatenated
        # word axis, halving the count of tiny fixed-overhead PE ops.
        state = {}

        def stage1(j):
            K = K4[j]
            KP = 256 if K > 128 else 128
            p = j // 2
            if j % 2 == 0:
                Ks = K4[2 * p] + K4[2 * p + 1]
                zs = zsp.tile([128, DC, Ks], bf16, tag="zs", name=f"zs{p}")
                ct16 = rsp.tile([1, Ks], bf16, tag="ct16", name=f"c6{p}")
                # rows 0..9 = pair logits (scalar, in tail), row 10 = counts
                lg_sb = rsp.tile([16, Ks], bf16, tag="lg", name=f"lg{p}")
                state[p] = (zs, ct16, lg_sb)
                off = 0
            else:
                zs, ct16, lg_sb = state[p]
                off = K4[j - 1]
            m_ts = []
            for c in range(TC[j]):
                m_t = mp.tile([128, KM], bf16, tag="m", name=f"m{j}_{c}")
                nc.vector.tensor_scalar(
                    m_t[:, 0:K], ck_t[:, 0:K], widT[:, c, j : j + 1], None,
                    op0=Alu.is_equal,
                )
                m_ts.append(m_t)
            ct_ps = ctp.tile([1, KM], f32, tag="ct", name=f"ct{j}")
            for h in range(2):
                zt = ztp.tile([128, DC // 2, KP], f32, tag="zt", name=f"zt{j}_{h}")
                # accumulation groups must be consecutive instructions:
                # keep the token-chunk loop innermost per PSUM region
                for jj in range(DC // 2):
                    dd = (h * (DC // 2) + jj) * 128
                    for c in range(TC[j]):
                        nc.tensor.matmul(
                            zt[:, jj, 0:K],
                            xs[(j, c)][:, dd : dd + 128],
                            m_ts[c][:, 0:K],
                            start=(c == 0),
                            stop=(c == TC[j] - 1),
                        )
                if h == 0:
                    for c in range(TC[j]):
                        nc.tensor.matmul(
                            ct_ps[:, 0:K], cr_t[:, 0:1], m_ts[c][:, 0:K],
                            start=(c == 0), stop=(c == TC[j] - 1),
                        )
                dst = zs[:, h * (DC // 2) : (h + 1) * (DC // 2), off : off + K]
                if h == 0:
                    nc.scalar.copy(dst, zt[:, :, 0:K])
                else:
                    nc.vector.tensor_copy(dst, zt[:, :, 0:K])
            # counts post: cnt' = max(cnt,1) in bf16; stacked as lg row 10
            # (via a tiny SBUF->SBUF DMA: engines can't write partition 10,
            # partition starts must be 32-aligned).  The same bf16 cnt' is
            # the bias-matmul operand, so (b*cnt')/cnt' cancels exactly.
            ct_sb = rsp.tile([1, KM], f32, tag="cts", name=f"cs{j}")
            nc.vector.tensor_scalar_max(ct_sb[:, 0:K], ct_ps[:, 0:K], 1.0)
            with nc.allow_low_precision(reason="bf16 cnt, tol 2e-2"):
                nc.vector.tensor_copy(ct16[:, off : off + K], ct_sb[:, 0:K])
            nc.sync.dma_start(
                lg_sb[NL : NL + 1, off : off + K], ct16[:, off : off + K]
            )

        def stage2_tail(p):
            zs, ct16, lg_sb = state.pop(p)
            Ks = K4[2 * p] + K4[2 * p + 1]
            # lg^T[l,k] = sum_d W^T[d,l] Z^T[d,k] + b[l]*cnt'[k], both slots
            lg_ps = lgp.tile([NL, 2 * KM], f32, tag="lgp", name=f"lp{p}")
            for jj in range(DC):
                nc.tensor.matmul(
                    lg_ps[:, 0:Ks], wt_v[:, jj, :], zs[:, jj, 0:Ks],
                    start=(jj == 0), stop=False,
                )
            nc.tensor.matmul(
                lg_ps[:, 0:Ks], b16[0:1, :], ct16[0:1, 0:Ks],
                start=False, stop=True,
            )
            nc.scalar.copy(lg_sb[0:NL, 0:Ks], lg_ps[:, 0:Ks])

            for jo in range(2):
                j = 2 * p + jo
                K = K4[j]
                off = 0 if jo == 0 else K4[j - 1]
                W2 = K // 2
                # 11-row transposes (logits + counts) in stride-2 pairs:
                # out partition q holds word 2q+c, fused scale+sigmoid, store
                row_out = obp.tile([128, 2, NL], f32, tag="row", name=f"ro{j}")
                recipT = obp.tile([128, 2], f32, tag="rT", name=f"rt{j}")
                nc.vector.tensor_copy(row_out[:], sigb2[:])
                lg_v = lg_sb[:].rearrange("l (q c) -> l q c", c=2)
                tp_ps = tpp.tile([128, 2, 16], bf16, tag="tp", name=f"tq{j}")
                for c in range(2):
                    nc.tensor.transpose(
                        tp_ps[0:W2, c, 0 : NL + 1],
                        lg_v[0 : NL + 1, off // 2 : off // 2 + W2, c],
                        ci_t[0 : NL + 1, 0 : NL + 1],
                    )
                nc.vector.reciprocal(
                    recipT[0:W2, :], tp_ps[0:W2, :, NL : NL + 1]
                )
                for c in range(2):
                    nc.scalar.activation(
                        row_out[0:W2, c, :], tp_ps[0:W2, c, 0:NL], Act.Sigmoid,
                        scale=recipT[0:W2, c : c + 1],
                    )
                nc.scalar.dma_start(
                    out_d[j, 0:256, :].rearrange("(q c) l -> q c l", c=2),
                    row_out[:],
                )

        for p in range(RPC // 2):
            stage1(2 * p)
            if p > 0:
                stage2_tail(p - 1)
            stage1(2 * p + 1)
        stage2_tail(RPC // 2 - 1)

    nc.compile()
    return nc


_NC_CACHE: dict = {}


def _prepare(input_mask, first_label_mask):
    order, TC, K4, CAP = _plan(input_mask, first_label_mask)
    if _NC_CACHE.get("key") != (TC, K4, CAP):
        _NC_CACHE["nc"] = _build_nc(TC, K4, CAP)
        _NC_CACHE["key"] = (TC, K4, CAP)
    _NC_CACHE["order"] = order
    _NC_CACHE["KM"] = max(K4)
    return _NC_CACHE["nc"]


def make_in_maps(token_features, input_mask, first_label_mask, W, b):
    _prepare(input_mask, first_label_mask)
    order, KM = _NC_CACHE["order"], _NC_CACHE["KM"]
    np_bf16 = mybir.dt.np(bf16)
    x = np.asarray(token_features, dtype=np.float32)
    im = np.asarray(input_mask, dtype=np.int32)
    fm = np.asarray(first_label_mask, dtype=np.int32)
    msk = np.concatenate([im, fm], axis=1)  # [B, 2S]
    # host-permuted W^T: wt[p, j*NL+l] = W[l, j*128+p]
    wt = np.ascontiguousarray(
        np.asarray(W, dtype=np.float32).T.reshape(DC, 128, NL)
        .transpose(1, 0, 2).reshape(128, DC * NL)
    )
    bb = np.ascontiguousarray(np.asarray(b, dtype=np.float32).reshape(1, NL))
    cb = np.zeros((128, 145 + KM), np.float32)
    cb[0:16, 0:16] = np.eye(16)
    cb[:, 16] = 1.0
    cb[:, 17:145] = 1.0
    cb[:, 145:] = np.arange(1, KM + 1, dtype=np.float32)[None, :]
    cb = np.ascontiguousarray(cb).astype(np_bf16)
    in_maps = []
    for i in range(N_CORES):
        rows = [order[j * N_CORES + i] for j in range(RPC)]
        in_maps.append(
            {
                "x": np.ascontiguousarray(x[rows]),
                "msk": np.ascontiguousarray(msk[rows]),
                "wt": wt, "b": bb, "cb": cb,
            }
        )
    return in_maps


def gather_out(res):
    order = _NC_CACHE["order"]
    out = np.empty((B, S, NL), np.float32)
    for i in range(N_CORES):
        o = res.results[i]["out"]
        for j in range(RPC):
            out[order[j * N_CORES + i]] = o[j]
    return out


def kernel(token_features, input_mask, first_label_mask, W, b):
    nc = _prepare(input_mask, first_label_mask)
    in_maps = make_in_maps(token_features, input_mask, first_label_mask, W, b)
    res = run_bass_kernel_spmd(nc, in_maps, list(range(N_CORES)))
    return gather_out(res)


if __name__ == "__main__":
    rng = np.random.default_rng(0)
    tf = rng.standard_normal((B, S, D), dtype=np.float32)
    lengths = rng.integers(16, S + 1, size=(B,))
    pos = np.arange(S)[None, :]
    im = (pos < lengths[:, None]).astype(np.int32)
    fm = ((rng.random((B, S)) < 0.4) & (im > 0)).astype(np.int32)
    fm[:, 1] = 1
    W = (rng.standard_normal((NL, D)) * 0.02).astype(np.float32)
    b = np.zeros(NL, np.float32)
    out = kernel(
        token_features=tf, input_mask=im, first_label_mask=fm, W=W, b=b
    )
    print(out.shape, out.dtype)
